# revision 34
# baseline (speedup 1.0000x reference)
"""Bass/Trainium2 kernel for nn_GCA (graph attention message passing layer).

Strategy v3 (8 NeuronCores, SPMD):
  - Nodes row-sharded: core m owns original nodes [5000m, 5000(m+1)), padded
    to 5120 slots per core (40 chunks x 128).
  - Phase A (per core, per 128-row chunk): LN1 (gamma/beta folded into the
    projection weights host-side) + fused q|k|v projection (one bf16 matmul,
    [128, 384]).  q rows -> q_loc DRAM table (bf16); k||v rows -> kv_bounce
    DRAM (fp8, 256 cols; the attention branch is ~1e-6 of the output norm,
    so fp8 is numerically free).
  - AllGather kv_bounce -> kv_full [40960, 256] fp8 (halo exchange).
  - Phase B: edges partitioned by destination chunk; per supergroup of 2
    chunks: dma_gather of per-edge q rows (by local dst) and k||v rows of
    edge sources (two tables so indices fit int16), spread over 4 SWDGE
    queues (a single queue serializes descriptor drain and costs ~1ms);
    per-edge score via DVE mult + segmented reduce (bf16), exp on ACT,
    un-normalized weights w (softmax over ALL edges per the reference, so
    aggregation is un-normalized); weighted v rows routed into a per-chunk
    PSUM accumulator [hd, dst] by matmul against a HOST-precomputed one-hot
    (fp8).  Pad slots gather a guaranteed-zero kv row -> w = exp(0) = 1
    exactly and an all-zero one-hot row; the exact global pad count is
    subtracted from the softmax denominator Z.  Z (per head) accumulated on
    DVE, AllReduced across cores, then folded into Wo's rows (one scale op).
  - Phase C (per chunk): o = aggT @ (Wo/z) + bo + residual, LN2 (folded into
    W1/b1), FFN via transposed-h trick (bias+ReLU fused on ACT, no mid-FFN
    transposes), + residual, write output rows (fp32).
"""

import math
import os

import numpy as np

import concourse.bass as bass
import concourse.bacc as bacc
import concourse.tile as tile
from concourse import mybir
from concourse import bass_utils

F32 = mybir.dt.float32
BF16 = mybir.dt.bfloat16
FP8 = mybir.dt.float8e4
I16 = mybir.dt.int16
AF = mybir.ActivationFunctionType
OP = mybir.AluOpType

M = 8            # cores
N = 40000        # nodes
C = 128          # channels
H = 8            # heads
D = 16           # head dim
E = 640000       # edges
FF = 512         # ffn dim
NPC = N // M     # 5000 nodes per core
NCH = 40         # chunks per core
NPAD = NCH * 128  # 5120 padded nodes per core
SPLIT = 8192     # table-B base row (int16 index headroom: 40960-8192-1 < 32768)
ZROW_A = NPC     # guaranteed-zero kv row for table A (core-0 pad row)
ZROW_B = 6 * NPAD + NPC  # guaranteed-zero kv row >= 32768 (core-6 pad row)
EPS = 1e-5
SG = 2           # chunks per supergroup (shared gathers/scatter)
NSG = NCH // SG
GB = 12          # edge tiles per DVE op group
KLIMIT = int(os.environ.get("KLIMIT", str(NCH)))
KABL = frozenset(x for x in os.environ.get("KABL", "").split(",") if x)
NQ = int(os.environ.get("KNQ", "4"))    # SWDGE queues for gathers
KVI = os.environ.get("KVI", "1") == "1"  # interleaved k||v gather
KF8 = os.environ.get("KF8", "1") == "1"  # fp8 kv table + one-hot


def _preprocess(x, edge_index):
    """Host-side sharding: per-core packed index tables + capacities."""
    src = np.asarray(edge_index[0], dtype=np.int64)
    dst = np.asarray(edge_index[1], dtype=np.int64)

    core = dst // NPC
    dst_loc = dst - core * NPC
    ch = dst_loc >> 7                    # 128-node chunk within core
    dst_rel = dst_loc & 127
    src_kv = (src // NPC) * NPAD + (src % NPC)
    hi = (src_kv >= 32768).astype(np.int64)

    grp = (core * NCH + ch) * 2 + hi     # [E]
    order = np.argsort(grp, kind="stable")
    grp_s = grp[order]
    uniq, first = np.unique(grp_s, return_index=True)
    pos = np.arange(E) - first[np.searchsorted(uniq, grp_s)]

    m_s = core[order]
    c_s = ch[order]
    hi_s = hi[order]
    dr_s = dst_rel[order]
    kv_s = src_kv[order]

    n_grp = np.zeros(M * NCH * 2, dtype=np.int64)
    np.add.at(n_grp, grp_s, 1)
    cap_a = max(128, math.ceil(n_grp[0::2].max() / 128) * 128)
    cap_b = max(128, math.ceil(n_grp[1::2].max() / 128) * 128)
    capq = cap_a + cap_b
    # slot within chunk: A edges at [0, cap_a), B edges at [cap_a, capq)
    slot = np.where(hi_s == 0, pos, cap_a + pos)

    # kv source indices (pads -> guaranteed-zero rows)
    idx_a = np.full((M, NCH, cap_a), ZROW_A, dtype=np.int16)
    idx_b = np.full((M, NCH, cap_b), ZROW_B - SPLIT, dtype=np.int16)
    lo_m = hi_s == 0
    idx_a[m_s[lo_m], c_s[lo_m], pos[lo_m]] = kv_s[lo_m].astype(np.int16)
    hi_m = ~lo_m
    idx_b[m_s[hi_m], c_s[hi_m], pos[hi_m]] = (kv_s[hi_m] - SPLIT).astype(np.int16)

    # q-gather indices per (chunk, a/b sub-table) slot
    # (q pads -> row 0, harmless since k=0)
    dloc = np.zeros((M, NCH, capq), dtype=np.int16)
    dloc[m_s, c_s, slot] = (c_s * 128 + dr_s).astype(np.int16)

    def wrap(a):  # [M, NSG, n] -> [M, NSG, 128, n//16] (16-wrap, x8 replicate)
        n = a.shape[-1]
        w = a.reshape(M, NSG, n // 16, 16).swapaxes(-1, -2)
        return np.tile(w, (1, 1, 8, 1))

    def sgcat(a):  # [M, NCH, n] -> [M, NSG, SG*n]: concat SG chunks' segments
        return a.reshape(M, NSG, SG * a.shape[-1])

    # supergroup tile order: [a(c0), a(c1), b(c0), b(c1)] -> q index
    # segments must follow the same slot order
    qseg = np.concatenate(
        [sgcat(dloc[:, :, :cap_a]), sgcat(dloc[:, :, cap_a:])], axis=-1)
    aseg = sgcat(idx_a)
    bseg = sgcat(idx_b)

    # packed per-supergroup index table: [q | a | b]
    idx = np.concatenate([wrap(qseg), wrap(aseg), wrap(bseg)], axis=-1)
    idx = np.ascontiguousarray(idx)

    # host-built one-hot routing matrices (pads: all-zero rows), in the
    # supergroup tile order [a(c0), a(c1), b(c0), b(c1)]
    ta, tb = cap_a // 128, cap_b // 128
    stt = SG * (ta + tb)
    oh = np.zeros((M, NSG, 128, stt, 128), dtype=np.float32)
    sg_i = c_s >> 1
    j_i = c_s & 1
    t_i = np.where(hi_s == 0, j_i * ta + (pos >> 7),
                   SG * ta + j_i * tb + (pos >> 7))
    oh[m_s, sg_i, pos & 127, t_i, dr_s] = 1.0
    import ml_dtypes
    ohdt = ml_dtypes.float8_e4m3 if KF8 else ml_dtypes.bfloat16
    oh = np.ascontiguousarray(
        oh.reshape(M, NSG, 128, stt * 128).astype(ohdt))

    x_loc = np.zeros((M, NPAD, C), dtype=np.float32)
    x_loc[:, :NPC] = np.asarray(x, dtype=np.float32).reshape(M, NPC, C)

    n_pads = float(M * NCH * capq - E)
    return x_loc, idx, oh, cap_a, cap_b, n_pads


def _build(cap_a, cap_b):
    ta, tb = cap_a // 128, cap_b // 128
    tt = ta + tb                      # tiles per chunk
    capq = cap_a + cap_b
    stt = SG * tt                     # tiles per supergroup
    scapq = SG * capq
    SGPACK = scapq + SG * (cap_a + cap_b)  # packed idx cols per supergroup
    nc = bacc.Bacc("TRN2", target_bir_lowering=False, debug=False,
                   num_devices=M, num_swdge_queues=NQ)

    din = {}
    for name, shape, dt in [
        ("x_loc", [NPAD, C], F32),
        ("wqkv", [C, 3 * C], BF16),      # LN1-folded q|k|v weights
        ("bqkv", [1, 3 * C], F32),       # LN1-folded q|k|v bias row
        ("bqkv39", [128, 3 * C], F32),   # same, pad rows zeroed (last chunk)
        ("wo", [C, C], BF16),
        ("bo", [1, C], F32),
        ("w1f", [C, FF], BF16),          # LN2-folded
        ("b1c", [C, 4], F32),            # LN2-folded b1, per-partition cols
        ("w2b", [C, FF], BF16),          # W2 blocks: [:, j*128:...] = W2[j*128:...]
        ("b2", [1, C], F32),
        ("ident", [128, 128], BF16),
        ("ones", [128, 1], F32),
        ("expand8", [H, 128], F32),
        ("pad8", [H, 1], F32),           # global pad count (per head row)
    ]:
        din[name] = nc.dram_tensor(name, shape, dt, kind="ExternalInput")
    din["idx"] = nc.dram_tensor("idx", [NSG, 128, SGPACK // 16], I16,
                                kind="ExternalInput")
    KVDT = FP8 if KF8 else BF16
    din["oh"] = nc.dram_tensor("oh", [NSG, 128, stt * 128], KVDT,
                               kind="ExternalInput")
    out_d = nc.dram_tensor("out", [NPAD, C], F32, kind="ExternalOutput")

    kv_bounce = nc.dram_tensor("kv_bounce", [NPAD, 2 * C], KVDT)
    kv_full = nc.dram_tensor("kv_full", [M * NPAD, 2 * C], KVDT,
                             addr_space="Shared")
    q_loc = nc.dram_tensor("q_loc", [NPAD, C], BF16)
    z_bounce = nc.dram_tensor("z_bounce", [128, H], F32)
    z_red = nc.dram_tensor("z_red", [128, H], F32, addr_space="Shared")

    with tile.TileContext(nc) as tc:
        with (
            tc.tile_pool(name="consts", bufs=1) as cp,
            tc.tile_pool(name="persist", bufs=1) as pp,
            tc.tile_pool(name="work", bufs=5) as wp,
            tc.tile_pool(name="gat", bufs=2) as gp,
            tc.tile_pool(name="wtp", bufs=2) as wtp,
            tc.tile_pool(name="ps_t", bufs=1, space="PSUM") as ps_t,
            tc.tile_pool(name="ps_mm", bufs=2, space="PSUM") as ps_mm,
            tc.tile_pool(name="ps_agg", bufs=2, space="PSUM") as ps_agg,
            tc.tile_pool(name="ps_h", bufs=2, space="PSUM") as ps_h,
            tc.tile_pool(name="ps_f", bufs=1, space="PSUM") as ps_f,
        ):
            # ---- constants ----
            def bload(name, cols, rows=128, dt=F32):
                t = cp.tile([rows, cols], dt, tag=name)
                src = din[name].ap()
                bc = bass.AP(tensor=src.tensor, offset=0,
                             ap=[[0, rows]] + list(src.ap[1:]))
                nc.sync.dma_start(out=t[:], in_=bc)
                return t

            bqkv_b = bload("bqkv", 3 * C)
            bo_b = bload("bo", C)
            b2_b = bload("b2", C)

            def cload(name, cols, dt=BF16, rows=128):
                t = cp.tile([rows, cols], dt, tag=name)
                nc.sync.dma_start(out=t[:], in_=din[name][:])
                return t

            wqkv_s = cload("wqkv", 3 * C)
            bqkv39_b = cload("bqkv39", 3 * C, dt=F32)
            wo_s = cload("wo", C)
            w1f_s = cload("w1f", FF)
            w2b_s = cload("w2b", FF)
            b1c_s = cload("b1c", 4, dt=F32)
            ident_s = cload("ident", 128)
            ones_s = cload("ones", 1, dt=F32)
            expand8_s = cload("expand8", 128, dt=F32, rows=H)
            pad8_s = cload("pad8", 1, dt=F32, rows=H)

            eps_t = cp.tile([128, 1], F32, tag="eps")
            nc.vector.memset(eps_t[:], EPS)
            z_acc = pp.tile([128, GB * H], F32, tag="z_acc")
            nc.vector.memset(z_acc[:], 0.0)

            x_tiles = [pp.tile([128, C], F32, name=f"x{i}", tag=f"x{i}")
                       for i in range(NCH)]
            a_tiles = [pp.tile([128, C], BF16, name=f"a{i}", tag=f"a{i}")
                       for i in range(NCH)]
            if KABL & {"noscat", "noinner", "nogather"} or KLIMIT < NCH:
                for t in a_tiles:
                    nc.vector.memset(t[:], 0.0)

            def ln_hat(x_t, out_t):
                """out = (x - mean) / sqrt(var + eps), bf16 (LN scale/bias
                folded into downstream weights)."""
                st = wp.tile([128, 6], F32, tag="ln_st")
                mv = wp.tile([128, 2], F32, tag="ln_mv")
                nc.vector.bn_stats(out=st[:], in_=x_t[:])
                nc.vector.bn_aggr(out=mv[:], in_=st[:])
                std = wp.tile([128, 1], F32, tag="ln_std")
                nc.scalar.activation(out=std[:], in_=mv[:, 1:2], func=AF.Sqrt,
                                     bias=eps_t[:], scale=1.0)
                nc.vector.reciprocal(out=std[:], in_=std[:])
                nc.vector.tensor_scalar(out=out_t[:], in0=x_t[:],
                                        scalar1=mv[:, 0:1], scalar2=std[:],
                                        op0=OP.subtract, op1=OP.mult)

            # ================= Phase A =================
            for i in range(NCH):
                x_t = x_tiles[i]
                nc.sync.dma_start(out=x_t[:],
                                  in_=din["x_loc"][i * 128:(i + 1) * 128, :])
                if "noA" in KABL:
                    continue
                xn = wp.tile([128, C], BF16, tag="xn")
                ln_hat(x_t, xn)
                xnT_p = ps_t.tile([128, C], BF16, tag="pt")
                nc.tensor.transpose(out=xnT_p[:], in_=xn[:], identity=ident_s[:])
                xnT = wp.tile([128, C], BF16, tag="xnT")
                nc.vector.tensor_copy(out=xnT[:], in_=xnT_p[:])

                qkv_p = ps_mm.tile([128, 3 * C], F32, tag="pmm")
                nc.tensor.matmul(out=qkv_p[:], lhsT=xnT[:], rhs=wqkv_s[:],
                                 start=True, stop=True)
                qt = wp.tile([128, C], BF16, tag="qt")
                kvt = wp.tile([128, 2 * C], KVDT, tag="kvt")
                # last chunk: masked bias so pad rows (locals >= 5000) stay 0
                bias = bqkv39_b if i == NCH - 1 else bqkv_b
                nc.vector.tensor_add(out=qt[:], in0=qkv_p[:, 0:C],
                                     in1=bias[:, 0:C])
                nc.vector.tensor_add(out=kvt[:], in0=qkv_p[:, C:3 * C],
                                     in1=bias[:, C:3 * C])
                nc.sync.dma_start(out=q_loc[i * 128:(i + 1) * 128, :],
                                  in_=qt[:])
                nc.sync.dma_start(out=kv_bounce[i * 128:(i + 1) * 128, :],
                                  in_=kvt[:])

            # ================= AllGather kv =================
            if "nocoll" not in KABL:
                nc.gpsimd.collective_compute(
                    "AllGather", OP.bypass,
                    replica_groups=[list(range(M))],
                    ins=[kv_bounce[:].opt()],
                    outs=[kv_full[:].opt()],
                )

            # ================= Phase B: edges =================
            o_a = scapq // 16                  # idx col offsets (per SG)
            o_b = o_a + SG * cap_a // 16
            for sgi in range(min(KLIMIT, NCH) // SG):
                idx_t = wp.tile([128, SGPACK // 16], I16, tag="idx")
                nc.sync.dma_start(out=idx_t[:], in_=din["idx"][sgi])
                oh_t = gp.tile([128, stt * 128], KVDT, tag="oh")
                nc.sync.dma_start(out=oh_t[:], in_=din["oh"][sgi])

                qg = gp.tile([128, stt * C], BF16, tag="qg")
                qg3 = qg[:].rearrange("p (t c) -> p t c", t=stt)
                if KVI:
                    kvg = gp.tile([128, stt * 2 * C], KVDT, tag="kvg")
                    kvg3 = kvg[:].rearrange("p (t c) -> p t c", t=stt)
                    kslc = lambda t0, bs: kvg3[:, t0:t0 + bs, 0:C]
                    vslc = lambda t0, bs: kvg3[:, t0:t0 + bs, C:2 * C]
                else:
                    kg = gp.tile([128, stt * C], KVDT, tag="kg")
                    kg3 = kg[:].rearrange("p (t c) -> p t c", t=stt)
                    vg = gp.tile([128, stt * C], KVDT, tag="vg")
                    vg3 = vg[:].rearrange("p (t c) -> p t c", t=stt)
                    kslc = lambda t0, bs: kg[:, t0 * C:(t0 + bs) * C]
                    vslc = lambda t0, bs: vg3[:, t0:t0 + bs, :]
                qn = [(sgi * 3 + i) % NQ for i in range(3)]
                if "nogather" in KABL:
                    nc.vector.memset(qg[:], 0.0)
                    if KVI:
                        nc.vector.memset(kvg[:], 0.0)
                    else:
                        nc.vector.memset(kg[:], 0.0)
                        nc.vector.memset(vg[:], 0.0)
                else:
                    nc.gpsimd.dma_gather(
                        out_ap=qg3[:, :, :], in_ap=q_loc[:],
                        idxs_ap=idx_t[:, 0:o_a],
                        num_idxs=scapq, num_idxs_reg=scapq,
                        elem_size=C, single_packet=False,
                        queue_num=qn[0])
                    for gi, (tbl, t0g, ng, oi) in enumerate(
                            ((kv_full, 0, SG * ta, o_a),
                             (kv_full[SPLIT:], SG * ta, SG * tb, o_b))):
                        n_idx = ng * 128
                        if KVI:
                            nc.gpsimd.dma_gather(
                                out_ap=kvg3[:, t0g:t0g + ng, :],
                                in_ap=tbl[:, :],
                                idxs_ap=idx_t[:, oi:oi + n_idx // 16],
                                num_idxs=n_idx, num_idxs_reg=n_idx,
                                elem_size=2 * C,
                                single_packet=False, queue_num=qn[1 + gi])
                        else:
                            nc.gpsimd.dma_gather(
                                out_ap=kg3[:, t0g:t0g + ng, :],
                                in_ap=tbl[:, 0:C],
                                idxs_ap=idx_t[:, oi:oi + n_idx // 16],
                                num_idxs=n_idx, num_idxs_reg=n_idx,
                                elem_size=C, elem_step=2 * C,
                                single_packet=False, queue_num=qn[1 + gi])
                            nc.gpsimd.dma_gather(
                                out_ap=vg3[:, t0g:t0g + ng, :],
                                in_ap=tbl[:, C:2 * C],
                                idxs_ap=idx_t[:, oi:oi + n_idx // 16],
                                num_idxs=n_idx, num_idxs_reg=n_idx,
                                elem_size=C, elem_step=2 * C,
                                single_packet=False, queue_num=qn[1 + gi])

                wt = wtp.tile([128, stt * C], KVDT, tag="wt")
                wt3 = wt[:].rearrange("p (t c) -> p t c", t=stt)
                if "noinner" in KABL:
                    nc.vector.memset(wt[:], 0.0)
                if "noinner" not in KABL:
                    t0 = 0
                    while t0 < stt:
                        bs = min(GB, stt - t0)
                        w = bs * 128
                        prod = wp.tile([128, GB * C], BF16, tag="prod")
                        nc.vector.tensor_mul(
                            out=prod[:, 0:w], in0=qg[:, t0 * C:t0 * C + w],
                            in1=kslc(t0, bs))
                        s8 = wp.tile([128, GB * H], BF16, tag="s8")
                        with nc.allow_low_precision("scores are O(0.1); "
                                                    "bf16 reduce is plenty"):
                            nc.vector.reduce_sum(
                                out=s8[:, 0:bs * H].rearrange(
                                    "p (b h) -> p b h", b=bs),
                                in_=prod[:, 0:w].rearrange(
                                    "p (b h d) -> p b h d", b=bs, h=H),
                                axis=mybir.AxisListType.X)
                        w8 = wp.tile([128, GB * H], BF16, tag="w8")
                        nc.scalar.activation(out=w8[:, 0:bs * H],
                                             in_=s8[:, 0:bs * H], func=AF.Exp,
                                             scale=1.0 / math.sqrt(D))
                        nc.vector.tensor_add(out=z_acc[:, 0:bs * H],
                                             in0=z_acc[:, 0:bs * H],
                                             in1=w8[:, 0:bs * H])
                        w8ap = w8[:, 0:bs * H]
                        w8b = bass.AP(
                            tensor=w8ap.tensor, offset=w8ap.offset,
                            ap=[list(w8ap.ap[0]), [H, bs], [1, H], [0, D]])
                        nc.vector.tensor_tensor(
                            out=wt3[:, t0:t0 + bs, :].rearrange(
                                "p b (h d) -> p b h d", h=H),
                            in0=vslc(t0, bs).rearrange(
                                "p b (h d) -> p b h d", h=H),
                            in1=w8b, op=OP.mult)
                        t0 += bs

                # scatter-add as matmul: per chunk, accumulate its tiles'
                # wt^T @ onehot into a PSUM agg (layout [hd, dst])
                if "noscat" not in KABL:
                    for j in range(SG):
                        tiles = ([j * ta + t for t in range(ta)] +
                                 [SG * ta + j * tb + t for t in range(tb)])
                        aggT_p = ps_agg.tile([128, 128], F32, tag="pagg")
                        for n, t in enumerate(tiles):
                            nc.tensor.matmul(
                                out=aggT_p[:],
                                lhsT=wt3[:, t, :],
                                rhs=oh_t[:, t * 128:(t + 1) * 128],
                                start=(n == 0), stop=(n == len(tiles) - 1))
                        nc.vector.tensor_copy(
                            out=a_tiles[sgi * SG + j][:], in_=aggT_p[:])

            # ================= Z reduce =================
            z8 = wp.tile([128, H], F32, tag="z8")
            za = z_acc[:]
            zsrc = bass.AP(tensor=za.tensor, offset=za.offset,
                           ap=[list(za.ap[0]), [1, H], [H, GB]])
            nc.vector.reduce_sum(out=z8[:], in_=zsrc,
                                 axis=mybir.AxisListType.X)
            nc.sync.dma_start(out=z_bounce[:], in_=z8[:])
            if "nocoll" not in KABL:
                nc.gpsimd.collective_compute(
                    "AllReduce", OP.add,
                    replica_groups=[list(range(M))],
                    ins=[z_bounce[:].opt()],
                    outs=[z_red[:].opt()],
                )
            z_red_s = wp.tile([128, H], F32, tag="z_red_s")
            nc.sync.dma_start(out=z_red_s[:], in_=z_red[:])
            zc_p = ps_t.tile([H, 1], F32, tag="pt")
            nc.tensor.matmul(out=zc_p[:], lhsT=z_red_s[:], rhs=ones_s[:],
                             start=True, stop=True)
            zc_s = wp.tile([H, 1], F32, tag="zc_s")
            nc.vector.tensor_scalar_sub(out=zc_s[:], in0=zc_p[:],
                                        scalar1=pad8_s[:])
            zhd_p = ps_t.tile([128, 1], F32, tag="pt")
            nc.tensor.matmul(out=zhd_p[:], lhsT=expand8_s[:], rhs=zc_s[:],
                             start=True, stop=True)
            zrec = cp.tile([128, 1], F32, tag="zrec")
            nc.vector.reciprocal(out=zrec[:], in_=zhd_p[:])
            woz = cp.tile([C, C], BF16, tag="woz")
            nc.vector.tensor_scalar_mul(out=woz[:], in0=wo_s[:],
                                        scalar1=zrec[:])

            # ================= Phase C =================
            for c in range(KLIMIT):
                o_p = ps_mm.tile([128, C], F32, tag="pmm")
                nc.tensor.matmul(out=o_p[:], lhsT=a_tiles[c][:], rhs=woz[:],
                                 start=True, stop=True)
                x1 = wp.tile([128, C], F32, tag="x1")
                nc.vector.tensor_add(out=x1[:], in0=o_p[:], in1=bo_b[:])
                nc.vector.tensor_add(out=x1[:], in0=x1[:], in1=x_tiles[c][:])

                xn2 = wp.tile([128, C], BF16, tag="xn2")
                ln_hat(x1, xn2)
                xn2T_p = ps_t.tile([128, C], BF16, tag="pt")
                nc.tensor.transpose(out=xn2T_p[:], in_=xn2[:],
                                    identity=ident_s[:])
                xn2T = wp.tile([128, C], BF16, tag="xn2T")
                nc.vector.tensor_copy(out=xn2T[:], in_=xn2T_p[:])

                hr = wp.tile([128, FF], BF16, tag="hr")
                for j in range(4):
                    h_p = ps_h.tile([128, C], F32, tag="ph")
                    nc.tensor.matmul(out=h_p[:],
                                     lhsT=w1f_s[:, j * C:(j + 1) * C],
                                     rhs=xn2T[:], start=True, stop=True)
                    nc.scalar.activation(out=hr[:, j * C:(j + 1) * C],
                                         in_=h_p[:], func=AF.Relu,
                                         bias=b1c_s[:, j:j + 1], scale=1.0)
                f_p = ps_f.tile([128, C], F32, tag="pf")
                for j in range(4):
                    nc.tensor.matmul(out=f_p[:],
                                     lhsT=hr[:, j * C:(j + 1) * C],
                                     rhs=w2b_s[:, j * C:(j + 1) * C],
                                     start=(j == 0), stop=(j == 3))
                fin = wp.tile([128, C], F32, tag="fin")
                nc.vector.tensor_add(out=fin[:], in0=f_p[:], in1=b2_b[:])
                nc.vector.tensor_add(out=fin[:], in0=fin[:], in1=x1[:])
                nc.sync.dma_start(out=out_d[c * 128:(c + 1) * 128, :],
                                  in_=fin[:])

    nc.compile()
    return nc


_CACHE = {}


def _make_in_maps(inputs, x_loc, idx, oh, n_pads):
    f32 = lambda a: np.ascontiguousarray(np.asarray(a, dtype=np.float32))

    g1 = f32(inputs["ln1_g"])
    b1 = f32(inputs["ln1_b"])
    g2 = f32(inputs["ln2_g"])
    b2l = f32(inputs["ln2_b"])
    Wq, Wk, Wv = f32(inputs["Wq"]), f32(inputs["Wk"]), f32(inputs["Wv"])
    W1, W2 = f32(inputs["W1"]), f32(inputs["W2"])

    wqkv = np.concatenate([g1[:, None] * Wq, g1[:, None] * Wk,
                           g1[:, None] * Wv], axis=1)
    bqkv = np.concatenate([b1 @ Wq + f32(inputs["bq"]),
                           b1 @ Wk + f32(inputs["bk"]),
                           b1 @ Wv + f32(inputs["bv"])])[None, :]
    w1f = g2[:, None] * W1
    b1c = (b2l @ W1 + f32(inputs["b1"])).reshape(4, C).T  # [C, 4]
    # w2b[:, j*128:(j+1)*128] = W2[j*128:(j+1)*128, :]
    w2b = np.concatenate([W2[j * C:(j + 1) * C, :] for j in range(4)], axis=1)

    expand8 = np.zeros((H, 128), dtype=np.float32)
    for h in range(H):
        expand8[h, h * D:(h + 1) * D] = 1.0

    bqkv39 = np.tile(bqkv, (128, 1))
    bqkv39[NPC - (NCH - 1) * 128:, C:] = 0.0

    import ml_dtypes
    bf16 = lambda a: np.ascontiguousarray(np.asarray(a).astype(
        ml_dtypes.bfloat16))
    shared = dict(
        wqkv=bf16(wqkv), bqkv=bqkv, bqkv39=np.ascontiguousarray(bqkv39),
        wo=bf16(inputs["Wo"]), bo=f32(inputs["bo"]).reshape(1, C),
        w1f=bf16(w1f), b1c=np.ascontiguousarray(b1c),
        w2b=bf16(w2b), b2=f32(inputs["b2"]).reshape(1, C),
        ident=bf16(np.eye(128, dtype=np.float32)),
        ones=np.ones((128, 1), dtype=np.float32),
        expand8=expand8,
        pad8=np.full((H, 1), n_pads, dtype=np.float32),
    )
    return [dict(shared, x_loc=x_loc[m], idx=idx[m], oh=oh[m])
            for m in range(M)]


def kernel(**inputs):
    x = np.asarray(inputs["x"], dtype=np.float32)
    edge_index = np.asarray(inputs["edge_index"])

    x_loc, idx, oh, cap_a, cap_b, n_pads = _preprocess(x, edge_index)

    key = (cap_a, cap_b)
    if key not in _CACHE:
        _CACHE[key] = _build(cap_a, cap_b)
    nc = _CACHE[key]

    in_maps = _make_in_maps(inputs, x_loc, idx, oh, n_pads)
    res = bass_utils.run_bass_kernel_spmd(nc, in_maps, core_ids=list(range(M)))
    out = np.concatenate([res.results[m]["out"][:NPC] for m in range(M)], axis=0)
    return out.astype(np.float32)
